# revision 45
# baseline (speedup 1.0000x reference)
"""BERT-NER forward (12-layer BERT-base + ragged compaction + 9-class head)
as a Bass/Tile kernel on 8 Trainium2 NeuronCores.

Sharding: data-parallel over batch. Core b processes sequence b (256 tokens).
Weights are replicated (pre-cast to bf16 on host); all GEMMs run in bf16 with
f32 PSUM accumulation.

Activation layout: feature-major X^T [768 (6 tiles x 128 partitions), 256
tokens]. Attention scores are computed transposed (S^T[k, q]) so the exp'd
score matrix feeds the AV matmul directly without transposing the attention
matrix. Softmax normalization, biases, LayerNorm statistics (partition-
direction reductions) and gamma/beta application are all folded into small
auxiliary matmuls. The valid_mask compaction is a host-built permutation
matrix applied as a final (exact) fp32 matmul over the per-token probability
rows.
"""
import os
import numpy as np
import ml_dtypes
from contextlib import ExitStack

import concourse.bass as bass
import concourse.bacc as bacc
import concourse.tile as tile
from concourse import mybir
from concourse.bass_utils import run_bass_kernel_spmd

F32 = mybir.dt.float32
BF16 = mybir.dt.bfloat16
I16 = mybir.dt.int16
AF = mybir.ActivationFunctionType
ALU = mybir.AluOpType

B, S, H, L, NH, FF, V, NL = 8, 256, 768, 12, 12, 3072, 30522, 9
D = H // NH           # 64
KT = H // 128         # 6 feature tiles
TT = S // 128         # 2 token tiles
FT = FF // 128        # 24 ff tiles
SCALE = 1.0 / np.sqrt(D)
EPS_U = (H * H) * 1e-12   # eps for u = 768^2 * var

N_LAYERS = int(os.environ.get("BERT_NL", str(L)))

# ---- params packing column layout (f32, feature-major [128, PC]) ----
PC_EMB_G = 0            # 6 cols
PC_EMB_B = 6            # 6 cols
PC_BIASK = 12           # 2 cols: attention additive mask bias per k position
PC_LAYER = 14           # per layer: bq(6), bk(6), bi(24), bo(6), bo2(6) = 48
PC_STRIDE = 48
PC_TOTAL = PC_LAYER + PC_STRIDE * L
NP = NH // 2            # 6 head pairs

TRACE = False           # set by test harness for profiling runs


def _ensure_ntff_hook():
    """The agent image's antenv lacks axon_hooks; shim it so trace=True can
    drive NTFF profiling through libaxon_pjrt.so (same ABI trn_boot uses)."""
    import sys, types
    if "antenv.axon_hooks" in sys.modules:
        return
    try:
        import antenv
        mod = types.ModuleType("antenv.axon_hooks")
        state = {"hook": None}
        mod.set_axon_ntff_profile_hook = lambda h: state.__setitem__("hook", h)
        mod.get_axon_ntff_profile_hook = lambda: state["hook"]
        sys.modules["antenv.axon_hooks"] = mod
        antenv.axon_hooks = mod
        from trn_agent_boot.trn_boot import _ntff_profile_via_ctypes
        mod.set_axon_ntff_profile_hook(
            _ntff_profile_via_ctypes("/opt/axon/libaxon_pjrt.so"))
    except Exception as e:  # profiling is best-effort
        print(f"ntff hook shim failed: {e}")


def _build_nc():
    nc = bacc.Bacc("TRN2", target_bir_lowering=False)

    # ---------------- DRAM tensors ----------------
    wq = nc.dram_tensor("wq", [L * H, H], BF16, kind="ExternalInput")
    wk = nc.dram_tensor("wk", [L * H, H], BF16, kind="ExternalInput")
    wv = nc.dram_tensor("wv", [L * H, H], BF16, kind="ExternalInput")
    wo = nc.dram_tensor("wo", [L * H, H], BF16, kind="ExternalInput")
    wi = nc.dram_tensor("wi", [L * H, FF], BF16, kind="ExternalInput")
    wo2 = nc.dram_tensor("wo2", [L * FF, H], BF16, kind="ExternalInput")
    word_emb = nc.dram_tensor("word_emb", [V, H], F32, kind="ExternalInput")
    pos_emb = nc.dram_tensor("pos_emb", [S, H], F32, kind="ExternalInput")
    type_emb = nc.dram_tensor("type_emb", [2, H], F32, kind="ExternalInput")
    clf_w = nc.dram_tensor("clf_w", [H, NL], F32, kind="ExternalInput")
    clf_b = nc.dram_tensor("clf_b", [1, NL], F32, kind="ExternalInput")
    params = nc.dram_tensor("params", [128, PC_TOTAL], F32, kind="ExternalInput")
    paramsT3 = nc.dram_tensor("paramsT3", [3, 2 * L * H], BF16, kind="ExternalInput")
    paramsW = nc.dram_tensor("paramsW", [65, 2 * L * H], BF16, kind="ExternalInput")
    selsum = nc.dram_tensor("selsum", [128, NH * NH], BF16, kind="ExternalInput")
    selpair = nc.dram_tensor("selpair", [NH, NP * 128], BF16, kind="ExternalInput")
    rows_bf = nc.dram_tensor("rows_bf", [1, 3 * L * H], BF16, kind="ExternalInput")
    idw = nc.dram_tensor("idw", [128, 16], I16, kind="ExternalInput")
    idt = nc.dram_tensor("idt", [128, 16], I16, kind="ExternalInput")
    permT = nc.dram_tensor("permT", [S, S], F32, kind="ExternalInput")
    padsel = nc.dram_tensor("padsel", [1, S], F32, kind="ExternalInput")
    ident = nc.dram_tensor("ident", [128, 128], F32, kind="ExternalInput")
    sumsel = nc.dram_tensor("sumsel", [128, 4], BF16, kind="ExternalInput")
    ones_col = nc.dram_tensor("ones_col", [128, 1], BF16, kind="ExternalInput")
    ones1_bf = nc.dram_tensor("ones1_bf", [1, 128], BF16, kind="ExternalInput")
    onesr_bf = nc.dram_tensor("onesr_bf", [1, S], BF16, kind="ExternalInput")
    ones1_f = nc.dram_tensor("ones1_f", [1, 128], F32, kind="ExternalInput")

    out = nc.dram_tensor("out", [S, NL], F32, kind="ExternalOutput")
    DEBUG = os.environ.get("BERT_DEBUG", "0") == "1"
    if DEBUG:
        d_emb = nc.dram_tensor("d_emb", [H, S], F32, kind="ExternalOutput")
        d_h1 = nc.dram_tensor("d_h1", [H, S], F32, kind="ExternalOutput")
        d_ff = nc.dram_tensor("d_ff", [128, S], BF16, kind="ExternalOutput")
        d_h2 = nc.dram_tensor("d_h2", [H, S], F32, kind="ExternalOutput")
        d_pr = nc.dram_tensor("d_pr", [S, NL], F32, kind="ExternalOutput")

    with tile.TileContext(nc) as tc, ExitStack() as ctx:
        ctx.enter_context(nc.allow_low_precision(
            reason="bf16 softmax/LN normalization factors; error budget analyzed"))
        const = ctx.enter_context(tc.tile_pool(name="const", bufs=1))
        act = ctx.enter_context(tc.tile_pool(name="act", bufs=1))
        small = ctx.enter_context(tc.tile_pool(name="small", bufs=1))
        ps = ctx.enter_context(tc.tile_pool(name="ps", bufs=1, space="PSUM"))

        # ---------------- constants / params ----------------
        # gather indices first so the embedding gathers aren't queued behind
        # the bulk const/weight DMA traffic
        idw_sb = const.tile([128, 16], I16, tag="idw", name="idw_sb")
        nc.sync.dma_start(idw_sb[:], idw[:, :])
        idt_sb = const.tile([128, 16], I16, tag="idt", name="idt_sb")
        nc.sync.dma_start(idt_sb[:], idt[:, :])
        ident_sb = const.tile([128, 128], F32, tag="ident", name="ident_sb")
        nc.sync.dma_start(ident_sb[:], ident[:, :])
        sumsel_sb = const.tile([128, 4], BF16, tag="sumsel", name="sumsel_sb")
        nc.sync.dma_start(sumsel_sb[:], sumsel[:, :])
        onescol_sb = const.tile([128, 1], BF16, tag="onescol", name="onescol_sb")
        nc.sync.dma_start(onescol_sb[:], ones_col[:, :])
        ones1b_sb = const.tile([1, 128], BF16, tag="ones1b", name="ones1b_sb")
        nc.sync.dma_start(ones1b_sb[:], ones1_bf[:, :])
        onesr_sb = const.tile([1, S], BF16, tag="onesr", name="onesr_sb")
        nc.sync.dma_start(onesr_sb[:], onesr_bf[:, :])
        ones1f_sb = const.tile([1, 128], F32, tag="ones1f", name="ones1f_sb")
        nc.sync.dma_start(ones1f_sb[:], ones1_f[:, :])
        eps_sb = const.tile([128, 1], F32, tag="eps", name="eps_sb")
        nc.vector.memset(eps_sb[:], float(EPS_U))
        # rhs3 for the fused LN-apply matmul: row 0 = [r | 0], row 32 =
        # [0 | s1*r], row 64 = [0 | -1]; zero/-1 regions are written once here
        rhs3 = const.tile([65, 2 * S], BF16, tag="rhs3", name="rhs3_sb")
        nc.vector.memset(rhs3[:], 0.0)
        nc.vector.memset(rhs3[64:65, S:2 * S], -1.0)
        params_sb = const.tile([128, PC_TOTAL], F32, tag="params", name="params_sb")
        nc.sync.dma_start(params_sb[:], params[:, :])
        selsum_sb = const.tile([128, NH * NH], BF16, tag="selsum", name="selsum_sb")
        nc.sync.dma_start(selsum_sb[:], selsum[:, :])
        selpair_sb = const.tile([NH, NP * 128], BF16, tag="selpair", name="selpair_sb")
        nc.sync.dma_start(selpair_sb[:], selpair[:, :])
        clfw_sb = const.tile([128, KT * NL], F32, tag="clfw", name="clfw_sb")
        for k in range(KT):
            nc.sync.dma_start(clfw_sb[:, k * NL:(k + 1) * NL],
                              clf_w[k * 128:(k + 1) * 128, :])
        clfb_sb = const.tile([1, NL], F32, tag="clfb", name="clfb_sb")
        nc.sync.dma_start(clfb_sb[:], clf_b[:, :])

        def pcol(c):
            return params_sb[:, c:c + 1]

        # ---------------- embeddings ----------------
        h = []
        for k in range(KT):
            t = act.tile([128, S], F32, tag=f"h{k}", name=f"h_{k}", bufs=2)
            h.append(t)
        with tc.tile_pool(name="emb", bufs=1) as emb:

            xt_sb = emb.tile([128, TT * H], F32, tag="xt", name="xt_sb")
            nc.gpsimd.dma_gather(
                out_ap=xt_sb.rearrange("p (c f) -> p c f", f=H),
                in_ap=word_emb[:, :],
                idxs_ap=idw_sb[:, :],
                num_idxs=S, num_idxs_reg=S, elem_size=H,
            )
            te_sb = emb.tile([128, TT * H], F32, tag="te", name="te_sb")
            nc.gpsimd.dma_gather(
                out_ap=te_sb.rearrange("p (c f) -> p c f", f=H),
                in_ap=type_emb[:, :],
                idxs_ap=idt_sb[:, :],
                num_idxs=S, num_idxs_reg=S, elem_size=H,
            )
            pe_sb = emb.tile([128, TT * H], F32, tag="pe", name="pe_sb")
            nc.sync.dma_start(
                pe_sb.rearrange("p (c f) -> p c f", f=H),
                pos_emb[:, :].rearrange("(c p) f -> p c f", p=128),
            )
            nc.vector.tensor_add(xt_sb[:], xt_sb[:], te_sb[:])
            nc.vector.tensor_add(xt_sb[:], xt_sb[:], pe_sb[:])

            # token-major LN (stats along free dim), then transpose to feat-major
            for c in range(TT):
                xc = xt_sb[:, c * H:(c + 1) * H]
                s1 = emb.tile([128, 1], F32, tag="s1", name=f"es1_{c}", bufs=2)
                nc.vector.reduce_sum(s1[:], xc, axis=mybir.AxisListType.X)
                sq = emb.tile([128, H], F32, tag="sq", name=f"esq_{c}", bufs=2)
                s2 = emb.tile([128, 1], F32, tag="s2", name=f"es2_{c}", bufs=2)
                nc.scalar.activation(sq[:], xc, AF.Square, accum_out=s2[:])
                t1 = emb.tile([128, 1], F32, tag="t1", name=f"et1_{c}", bufs=2)
                nc.scalar.activation(t1[:], s1[:], AF.Square)
                u = emb.tile([128, 1], F32, tag="u", name=f"eu_{c}", bufs=2)
                nc.vector.scalar_tensor_tensor(u[:], s2[:], float(H), t1[:],
                                               op0=ALU.mult, op1=ALU.subtract)
                qq = emb.tile([128, 1], F32, tag="qq", name=f"eqq_{c}", bufs=2)
                nc.scalar.activation(qq[:], u[:], AF.Sqrt, bias=eps_sb[:])
                rr = emb.tile([128, 1], F32, tag="rr", name=f"err_{c}", bufs=2)
                nc.vector.reciprocal(rr[:], qq[:])
                scale0 = emb.tile([128, 1], F32, tag="scale0", name=f"esc_{c}", bufs=2)
                nc.scalar.activation(scale0[:], rr[:], AF.Copy, scale=float(H))
                bias0 = emb.tile([128, 1], F32, tag="bias0", name=f"ebi_{c}", bufs=2)
                nc.vector.scalar_tensor_tensor(bias0[:], s1[:], -1.0, rr[:],
                                               op0=ALU.mult, op1=ALU.mult)
                xln = emb.tile([128, H], F32, tag="xln", name=f"exln_{c}", bufs=2)
                nc.scalar.activation(xln[:], xc, AF.Identity,
                                     bias=bias0[:], scale=scale0[:])
                # transpose 128x128 blocks to feature-major, apply emb gamma/beta
                for k in range(KT):
                    tp = ps.tile([128, 128], F32, tag="mm", name=f"etp_{c}_{k}", bufs=4)
                    nc.tensor.transpose(tp[:], xln[:, k * 128:(k + 1) * 128],
                                        ident_sb[:])
                    nc.scalar.activation(h[k][:, c * 128:(c + 1) * 128], tp[:],
                                         AF.Identity,
                                         bias=pcol(PC_EMB_B + k),
                                         scale=pcol(PC_EMB_G + k))

        hln = []
        for k in range(KT):
            t = act.tile([128, S], BF16, tag=f"hb{k}", name=f"hb_{k}", bufs=2)
            nc.scalar.copy(t[:], h[k][:])
            hln.append(t)
        if DEBUG:
            for k in range(KT):
                nc.sync.dma_start(d_emb[k * 128:(k + 1) * 128, :], h[k][:])

        wpool = ctx.enter_context(tc.tile_pool(name="wpool", bufs=1))

        # ---------------- transformer layers ----------------
        def layernorm(h_in, hcsq, which, l):
            """which: 0 -> ln1, 1 -> ln2. Returns (h_new_tiles, hln_bf_tiles).

            Stats: one [hb|sq] concat tile per k -> 6 N=512 matmuls into a
            single [1, 2S] PSUM tile (s1 | s2). Apply: one K=3 matmul per k
            produces [sg | mb] in one PSUM bank."""
            base_t = (l * 2 + which) * H
            ptW = small.tile([65, H], BF16, tag="ptW", name=f"ptW_{l}_{which}", bufs=2)
            nc.sync.dma_start(ptW[:], paramsW[:, base_t:base_t + H])
            s12 = ps.tile([1, 2 * S], F32, tag="sc", name=f"lns_{l}_{which}", bufs=2)
            # dummy op preloads the Abs_reciprocal_sqrt activation table while
            # the stats matmuls run, keeping the 1.3us table load off the
            # serial LN chain
            dum = small.tile([1, 1], F32, tag="dummy", name=f"dum_{l}_{which}",
                             bufs=2)
            nc.scalar.activation(dum[:], eps_sb[0:1, :], AF.Abs_reciprocal_sqrt)
            for k in range(KT):
                nc.tensor.matmul(s12[:], onescol_sb[:], hcsq[k][:],
                                 start=(k == 0), stop=(k == KT - 1))
            s1sb = small.tile([1, S], F32, tag="s1sb", name=f"s1sb_{l}_{which}",
                              bufs=2)
            nc.vector.tensor_copy(s1sb[:], s12[0:1, 0:S])
            t1 = small.tile([1, S], F32, tag="t1", name=f"t1_{l}_{which}", bufs=2)
            nc.vector.tensor_mul(t1[:], s1sb[:], s1sb[:])
            u = small.tile([1, S], F32, tag="u", name=f"u_{l}_{which}", bufs=2)
            nc.vector.scalar_tensor_tensor(u[:], s12[0:1, S:2 * S], float(H), t1[:],
                                           op0=ALU.mult, op1=ALU.subtract)
            # r lands directly in rhs3 row 0; s1*r in row 32 (base-32 write
            # is legal); row 64 holds the constant -1
            nc.scalar.activation(rhs3[0:1, 0:S], u[:], AF.Abs_reciprocal_sqrt,
                                 bias=eps_sb[0:1, :])
            nc.vector.tensor_mul(rhs3[32:33, S:2 * S], s1sb[:], rhs3[0:1, 0:S])
            h_new, hb_new = [], []
            for k in range(KT):
                sgmb = ps.tile([128, 2 * S], F32, tag="sc",
                               name=f"sgmb_{l}_{which}_{k}", bufs=2)
                nc.tensor.matmul(sgmb[:], ptW[:, k * 128:(k + 1) * 128], rhs3[:])
                tmp = small.tile([128, S], F32, tag="lntmp",
                                 name=f"lnt_{l}_{which}_{k}", bufs=2)
                nc.vector.tensor_mul(tmp[:], h_in[k][:], sgmb[:, 0:S])
                hn = act.tile([128, S], F32, tag=f"h{k}", name=f"h_{l}_{which}_{k}",
                              bufs=2)
                nc.vector.tensor_sub(hn[:], tmp[:], sgmb[:, S:2 * S])
                hb = act.tile([128, S], BF16, tag=f"hb{k}",
                              name=f"hbn_{l}_{which}_{k}", bufs=2)
                nc.scalar.copy(hb[:], hn[:])
                h_new.append(hn)
                hb_new.append(hb)
            # preload the table the NEXT scalar table-function needs (gelu
            # after ln1, next layer's exp after ln2) AFTER the hb casts so the
            # load hides under the following GEMM phase
            dum2 = small.tile([1, 1], F32, tag="dummy", name=f"dum2_{l}_{which}",
                              bufs=2)
            nc.scalar.activation(dum2[:], eps_sb[0:1, :],
                                 AF.Gelu if which == 0 else AF.Exp)
            return h_new, hb_new

        for l in range(N_LAYERS):
            # ---- weight panels ----
            wq_p, wk_p, wv_p, wo_p = [], [], [], []
            for k in range(KT):
                t = wpool.tile([128, H], BF16, tag="wp768", name=f"wq_{l}_{k}", bufs=36)
                nc.sync.dma_start(t[:], wq[l * H + k * 128: l * H + (k + 1) * 128, :])
                wq_p.append(t)
            for k in range(KT):
                t = wpool.tile([128, H], BF16, tag="wp768", name=f"wk_{l}_{k}", bufs=36)
                nc.sync.dma_start(t[:], wk[l * H + k * 128: l * H + (k + 1) * 128, :])
                wk_p.append(t)
            for k in range(KT):
                t = wpool.tile([128, H], BF16, tag="wp768", name=f"wv_{l}_{k}", bufs=36)
                nc.sync.dma_start(t[:], wv[l * H + k * 128: l * H + (k + 1) * 128, :])
                wv_p.append(t)

            pbase = PC_LAYER + PC_STRIDE * l

            # ---- Q^T, K^T per feature-pair, scores + exp interleaved so the
            # scalar-engine exps hide under the Q/K GEMMs ----
            q_bf, k_bf = [], []
            e_bf = [[None] * TT for _ in range(NP)]
            for m in range(KT):
                accq = ps.tile([128, S], F32, tag="mm", name=f"q_{l}_{m}", bufs=4)
                for k in range(KT):
                    nc.tensor.matmul(accq[:], wq_p[k][:, m * 128:(m + 1) * 128],
                                     hln[k][:], start=(k == 0), stop=(k == KT - 1))
                qo = act.tile([128, S], BF16, tag=f"qk0_{m}",
                              name=f"qko_{l}_{m}", bufs=1)
                nc.vector.tensor_scalar_add(qo[:], accq[:], pcol(pbase + m))
                q_bf.append(qo)
                acck = ps.tile([128, S], F32, tag="mm", name=f"k_{l}_{m}", bufs=4)
                for k in range(KT):
                    nc.tensor.matmul(acck[:], wk_p[k][:, m * 128:(m + 1) * 128],
                                     hln[k][:], start=(k == 0), stop=(k == KT - 1))
                ko = act.tile([128, S], BF16, tag=f"qk1_{m}",
                              name=f"qko1_{l}_{m}", bufs=1)
                nc.vector.tensor_scalar_add(ko[:], acck[:], pcol(pbase + 6 + m))
                k_bf.append(ko)
                # scores for head pair m: two heads concurrent via row groups.
                # One PSUM bank per accumulation group (start zeroes the whole
                # bank), exp'd halves land in one wide SBUF tile.
                for kt in range(TT):
                    e = act.tile([128, 2 * S], BF16, tag=f"e{m}_{kt}",
                                 name=f"e_{l}_{m}_{kt}", bufs=1)
                    for sub in range(2):
                        sc = ps.tile([128, S], F32, tag="mm",
                                     name=f"sc_{l}_{m}_{kt}_{sub}", bufs=4)
                        nc.tensor.matmul(
                            sc[:],
                            ko[sub * D:(sub + 1) * D, kt * 128:(kt + 1) * 128],
                            qo[sub * D:(sub + 1) * D, :])
                        nc.scalar.activation(e[:, sub * S:(sub + 1) * S], sc[:],
                                             AF.Exp, bias=pcol(PC_BIASK + kt),
                                             scale=float(SCALE))
                    e_bf[m][kt] = e

            # ---- V (X^T stationary -> token-major V); bv folded into bo ----
            v_bf = []
            for mt in range(TT):
                accA = ps.tile([128, 512], F32, tag="sc", name=f"vA_{l}_{mt}", bufs=2)
                accB = ps.tile([128, S], F32, tag="mm", name=f"vB_{l}_{mt}", bufs=4)
                for k in range(KT):
                    nc.tensor.matmul(accA[:],
                                     hln[k][:, mt * 128:(mt + 1) * 128],
                                     wv_p[k][:, 0:512],
                                     start=(k == 0), stop=(k == KT - 1))
                    nc.tensor.matmul(accB[:],
                                     hln[k][:, mt * 128:(mt + 1) * 128],
                                     wv_p[k][:, 512:H],
                                     start=(k == 0), stop=(k == KT - 1))
                vb = act.tile([128, H], BF16, tag=f"v{mt}", name=f"vb_{l}_{mt}", bufs=1)
                nc.scalar.copy(vb[:, 0:512], accA[:])
                nc.scalar.copy(vb[:, 512:H], accB[:])
                v_bf.append(vb)

            # ---- softmax sums first (selector-accumulated into one [NH, S]
            # tile) so the reciprocal runs on DVE while PE does the AV matmuls ----
            sums_ps = ps.tile([NH, S], F32, tag="sc", name=f"sums_{l}", bufs=2)
            for pair in range(NP):
                for kt in range(TT):
                    for sub in range(2):
                        hh = 2 * pair + sub
                        nc.tensor.matmul(
                            sums_ps[:],
                            selsum_sb[:, hh * NH:(hh + 1) * NH],
                            e_bf[pair][kt][:, sub * S:(sub + 1) * S],
                            start=(pair == 0 and kt == 0 and sub == 0),
                            stop=(pair == NP - 1 and kt == TT - 1 and sub == 1))
            rc_bf = small.tile([NH, S], BF16, tag="rcb", name=f"rcb_{l}", bufs=2)
            nc.vector.reciprocal(rc_bf[:], sums_ps[:])

            # unnormalized AV (PSUM evac on scalar; DVE is busy with the
            # reciprocal)
            cb0 = []
            for pair in range(NP):
                c0 = act.tile([128, S], BF16, tag=f"cb0_{pair}",
                              name=f"cb0_{l}_{pair}", bufs=1)
                for sub in range(2):
                    hh = 2 * pair + sub
                    cps = ps.tile([64, S], F32, tag="cps",
                                  name=f"cps_{l}_{pair}_{sub}", bufs=2)
                    for kt in range(TT):
                        nc.tensor.matmul(
                            cps[:],
                            v_bf[kt][:, hh * D:(hh + 1) * D],
                            e_bf[pair][kt][:, sub * S:(sub + 1) * S],
                            start=(kt == 0), stop=(kt == TT - 1))
                    nc.scalar.copy(c0[sub * D:(sub + 1) * D, :], cps[:])
                cb0.append(c0)

            ctx_bf = []
            for pair in range(NP):
                rb = ps.tile([128, S], F32, tag="cps", name=f"rb_{l}_{pair}", bufs=2)
                nc.tensor.matmul(rb[:], selpair_sb[:, pair * 128:(pair + 1) * 128],
                                 rc_bf[:])
                rbs = small.tile([128, S], BF16, tag="rbs",
                                 name=f"rbs_{l}_{pair}", bufs=3)
                nc.vector.tensor_copy(rbs[:], rb[:])
                cb = act.tile([128, S], BF16, tag=f"ctx{pair}",
                              name=f"ctx_{l}_{pair}", bufs=1)
                nc.vector.tensor_mul(cb[:], cb0[pair][:], rbs[:])
                ctx_bf.append(cb)

            # ---- O projection + residual ----
            for k in range(KT):
                t = wpool.tile([128, H], BF16, tag="wp768", name=f"wo_{l}_{k}", bufs=36)
                nc.sync.dma_start(t[:], wo[l * H + k * 128: l * H + (k + 1) * 128, :])
                wo_p.append(t)
            h_res, hcsq1 = [], []
            for m in range(KT):
                acc = ps.tile([128, S], F32, tag="mm", name=f"o_{l}_{m}", bufs=4)
                for k in range(KT):
                    nc.tensor.matmul(acc[:], wo_p[k][:, m * 128:(m + 1) * 128],
                                     ctx_bf[k][:], start=(k == 0),
                                     stop=(k == KT - 1))
                hr = small.tile([128, S], F32, tag=f"hr{m}", name=f"hr_{l}_{m}", bufs=1)
                nc.vector.scalar_tensor_tensor(hr[:], acc[:], pcol(pbase + 36 + m),
                                               h[m][:], op0=ALU.add, op1=ALU.add)
                h_res.append(hr)
                hc = small.tile([128, 2 * S], BF16, tag="hcsq",
                                name=f"hcsq_{l}_0_{m}", bufs=7)
                nc.vector.tensor_copy(hc[:, 0:S], hr[:])
                nc.scalar.activation(hc[:, S:2 * S], hr[:], AF.Square)
                hcsq1.append(hc)

            h, hln = layernorm(h_res, hcsq1, 0, l)
            if DEBUG and l == 0:
                for k in range(KT):
                    nc.sync.dma_start(d_h1[k * 128:(k + 1) * 128, :], h[k][:])

            # ---- FF1 (Wi stationary) + gelu ----
            wi_p = []
            for k in range(KT):
                t = wpool.tile([128, FF], BF16, tag="wp3072", name=f"wi_{l}_{k}",
                               bufs=7)
                nc.sync.dma_start(t[:], wi[l * H + k * 128: l * H + (k + 1) * 128, :])
                wi_p.append(t)
            ff_bf = []
            for m in range(FT):
                acc = ps.tile([128, S], F32, tag="mm", name=f"ff1_{l}_{m}", bufs=4)
                for k in range(KT):
                    nc.tensor.matmul(acc[:], wi_p[k][:, m * 128:(m + 1) * 128],
                                     hln[k][:], start=(k == 0), stop=(k == KT - 1))
                fb = act.tile([128, S], BF16, tag=f"ff{m}", name=f"ff_{l}_{m}", bufs=1)
                nc.scalar.activation(fb[:], acc[:], AF.Gelu,
                                     bias=pcol(pbase + 12 + m))
                ff_bf.append(fb)

            if DEBUG and l == 0:
                nc.sync.dma_start(d_ff[:, :], ff_bf[0][:])

            # ---- FF2 (m-outer, K-contiguous per output tile) + residual ----
            wo2_p = []
            for k in range(FT):
                t = wpool.tile([128, H], BF16, tag="wp768", name=f"wo2_{l}_{k}",
                               bufs=36)
                nc.sync.dma_start(t[:], wo2[l * FF + k * 128: l * FF + (k + 1) * 128, :])
                wo2_p.append(t)
            h_res2, hcsq2 = [], []
            for m in range(KT):
                acc = ps.tile([128, S], F32, tag="mm", name=f"ff2_{l}_{m}", bufs=4)
                for k in range(FT):
                    nc.tensor.matmul(acc[:], wo2_p[k][:, m * 128:(m + 1) * 128],
                                     ff_bf[k][:], start=(k == 0),
                                     stop=(k == FT - 1))
                hr = small.tile([128, S], F32, tag=f"hr{m}", name=f"hr2_{l}_{m}",
                                bufs=1)
                nc.vector.scalar_tensor_tensor(hr[:], acc[:], pcol(pbase + 42 + m),
                                               h[m][:], op0=ALU.add, op1=ALU.add)
                h_res2.append(hr)
                hc = small.tile([128, 2 * S], BF16, tag="hcsq",
                                name=f"hcsq_{l}_1_{m}", bufs=7)
                nc.vector.tensor_copy(hc[:, 0:S], hr[:])
                nc.scalar.activation(hc[:, S:2 * S], hr[:], AF.Square)
                hcsq2.append(hc)

            h, hln = layernorm(h_res2, hcsq2, 1, l)
            if DEBUG and l == 0:
                for k in range(KT):
                    nc.sync.dma_start(d_h2[k * 128:(k + 1) * 128, :], h[k][:])

        # ---------------- classifier + softmax + compaction ----------------
        permT_sb = []
        for kt in range(TT):
            for mt in range(TT):
                t = small.tile([128, 128], F32, tag=f"permT{kt}_{mt}",
                               name=f"permT_{kt}_{mt}", bufs=1)
                nc.sync.dma_start(t[:], permT[kt * 128:(kt + 1) * 128,
                                              mt * 128:(mt + 1) * 128])
                permT_sb.append(t)
        padsel_sb = small.tile([1, S], F32, tag="padsel", name="padsel_sb", bufs=1)
        nc.sync.dma_start(padsel_sb[:], padsel[:, :])

        # pad row = softmax(clf_b)
        pmx = small.tile([1, 1], F32, tag="pmx", name="pmx", bufs=1)
        nc.vector.reduce_max(pmx[:], clfb_sb[:], axis=mybir.AxisListType.X,
                             negate=True)
        pex = small.tile([1, NL], F32, tag="pex", name="pex", bufs=1)
        psm = small.tile([1, 1], F32, tag="psm", name="psm", bufs=1)
        nc.scalar.activation(pex[:], clfb_sb[:], AF.Exp, bias=pmx[:],
                             accum_out=psm[:])
        prs = small.tile([1, 1], F32, tag="prs", name="prs", bufs=1)
        nc.vector.reciprocal(prs[:], psm[:])
        ppr = small.tile([1, NL], F32, tag="ppr", name="ppr", bufs=1)
        nc.vector.tensor_scalar_mul(ppr[:], pex[:], prs[:])

        probs = []
        for mt in range(TT):
            acc = ps.tile([128, NL], F32, tag="mm", name=f"clf_{mt}", bufs=4)
            for k in range(KT):
                nc.tensor.matmul(acc[:], h[k][:, mt * 128:(mt + 1) * 128],
                                 clfw_sb[:, k * NL:(k + 1) * NL],
                                 start=(k == 0), stop=False)
            nc.tensor.matmul(acc[:], ones1f_sb[:], clfb_sb[:],
                             start=False, stop=True)
            mx = small.tile([128, 1], F32, tag="mx", name=f"mx_{mt}", bufs=2)
            nc.vector.reduce_max(mx[:], acc[:], axis=mybir.AxisListType.X,
                                 negate=True)
            ex = small.tile([128, NL], F32, tag="ex", name=f"ex_{mt}", bufs=2)
            sm = small.tile([128, 1], F32, tag="sm", name=f"sm_{mt}", bufs=2)
            nc.scalar.activation(ex[:], acc[:], AF.Exp, bias=mx[:],
                                 accum_out=sm[:])
            rs = small.tile([128, 1], F32, tag="rs", name=f"rs_{mt}", bufs=2)
            nc.vector.reciprocal(rs[:], sm[:])
            pr = small.tile([128, NL], F32, tag=f"pr{mt}", name=f"pr_{mt}", bufs=1)
            nc.vector.tensor_scalar_mul(pr[:], ex[:], rs[:])
            probs.append(pr)

        if DEBUG:
            for mt in range(TT):
                nc.sync.dma_start(d_pr[mt * 128:(mt + 1) * 128, :], probs[mt][:])

        # compacted output rows: out[i] = probs[order[i]] (i < count) else pad
        for mt in range(TT):
            acc = ps.tile([128, NL], F32, tag="mm", name=f"cmp_{mt}", bufs=4)
            for kt in range(TT):
                nc.tensor.matmul(acc[:], permT_sb[kt * TT + mt][:], probs[kt][:],
                                 start=(kt == 0), stop=False)
            nc.tensor.matmul(acc[:], padsel_sb[0:1, mt * 128:(mt + 1) * 128],
                             ppr[:], start=False, stop=True)
            osb = small.tile([128, NL], F32, tag=f"osb{mt}", name=f"osb_{mt}", bufs=1)
            nc.scalar.copy(osb[:], acc[:])
            nc.sync.dma_start(out[mt * 128:(mt + 1) * 128, :], osb[:])

    nc.finalize()
    return nc


_NC_CACHE = {}


def _get_nc():
    key = N_LAYERS
    if key not in _NC_CACHE:
        _NC_CACHE[key] = _build_nc()
    return _NC_CACHE[key]


def _pack_host(inputs):
    """Builds per-core in_maps (host-side sharding + descriptor prep)."""
    f32 = np.float32
    bf16 = ml_dtypes.bfloat16

    Wq = np.ascontiguousarray(inputs["Wq"].astype(bf16).reshape(L * H, H))
    Wk = np.ascontiguousarray(inputs["Wk"].astype(bf16).reshape(L * H, H))
    Wv = np.ascontiguousarray(inputs["Wv"].astype(bf16).reshape(L * H, H))
    Wo = np.ascontiguousarray(inputs["Wo"].astype(bf16).reshape(L * H, H))
    Wi = np.ascontiguousarray(inputs["Wi"].astype(bf16).reshape(L * H, FF))
    Wo2 = np.ascontiguousarray(inputs["Wo2"].astype(bf16).reshape(L * FF, H))

    # params (feature-major per-partition columns)
    params = np.zeros((128, PC_TOTAL), f32)
    params[:, PC_EMB_G:PC_EMB_G + 6] = inputs["emb_ln_g"].reshape(6, 128).T
    params[:, PC_EMB_B:PC_EMB_B + 6] = inputs["emb_ln_b"].reshape(6, 128).T
    for l in range(L):
        base = PC_LAYER + PC_STRIDE * l
        params[:, base:base + 6] = inputs["bq"][l].reshape(6, 128).T
        params[:, base + 6:base + 12] = inputs["bk"][l].reshape(6, 128).T
        params[:, base + 12:base + 36] = inputs["bi"][l].reshape(24, 128).T
        bo_folded = (inputs["bo"][l] + inputs["bv"][l].astype(np.float64) @
                     inputs["Wo"][l].astype(np.float64)).astype(f32)
        params[:, base + 36:base + 42] = bo_folded.reshape(6, 128).T
        params[:, base + 42:base + 48] = inputs["bo2"][l].reshape(6, 128).T

    # paramsT3 rows: [768*gamma, gamma, beta]
    paramsT3 = np.zeros((3, 2 * L * H), f32)
    for l in range(L):
        for which, (g, b) in enumerate(
                [(inputs["ln1_g"][l], inputs["ln1_b"][l]),
                 (inputs["ln2_g"][l], inputs["ln2_b"][l])]):
            c0 = (l * 2 + which) * H
            paramsT3[0, c0:c0 + H] = g * float(H)
            paramsT3[1, c0:c0 + H] = g
            paramsT3[2, c0:c0 + H] = b
    paramsT3 = paramsT3.astype(bf16)
    # K=65 padded LN-apply weights: row 0 = H*gamma, row 32 = gamma, row 64 =
    # beta (other rows zero, matching rhs3's sparse row layout)
    paramsW = np.zeros((65, 2 * L * H), f32)
    paramsW[0] = np.asarray(paramsT3[0], dtype=f32)
    paramsW[32] = np.asarray(paramsT3[1], dtype=f32)
    paramsW[64] = np.asarray(paramsT3[2], dtype=f32)
    paramsW = paramsW.astype(bf16)

    # softmax-sum selector: block hh is [128, NH] with column hh all-ones
    selsum = np.zeros((128, NH * NH), f32)
    for hh in range(NH):
        selsum[:, hh * NH + hh] = 1.0
    selsum = selsum.astype(bf16)
    # reciprocal broadcast selector: block `pair` maps rc rows (2p, 2p+1)
    # to output partitions [0:64), [64:128)
    selpair = np.zeros((NH, (NH // 2) * 128), f32)
    for pair in range(NH // 2):
        selpair[2 * pair, pair * 128:pair * 128 + 64] = 1.0
        selpair[2 * pair + 1, pair * 128 + 64:pair * 128 + 128] = 1.0
    selpair = selpair.astype(bf16)

    # rows_bf: [unused | bo' | bo2] blocks per layer, single partition row.
    # bv is folded into bo: attention ctx rows are normalized (sum to 1), so
    # ctx_with_bias = ctx_norm + 1*bv and (ctx+1*bv) @ Wo = ctx @ Wo + bv @ Wo.
    rows = np.zeros((1, 3 * L * H), f32)
    for l in range(L):
        bo_folded = inputs["bo"][l] + inputs["bv"][l].astype(np.float64) @ \
            inputs["Wo"][l].astype(np.float64)
        rows[0, 3 * l * H + 1 * H:3 * l * H + 2 * H] = bo_folded.astype(f32)
        rows[0, 3 * l * H + 2 * H:3 * l * H + 3 * H] = inputs["bo2"][l]
    rows = rows.astype(bf16)

    ident = np.eye(128, dtype=f32)
    sumsel = np.zeros((128, 4), f32)
    sumsel[:, 0] = 1.0   # S1 -> row 0
    sumsel[:, 3] = 1.0   # S2 -> row 1
    sumsel = sumsel.astype(bf16)
    ones_col = np.ones((128, 1), bf16)
    ones1b = np.ones((1, 128), bf16)
    onesr = np.ones((1, S), bf16)
    ones1f = np.ones((1, 128), f32)

    word_emb = np.ascontiguousarray(inputs["word_emb"].astype(f32))
    pos_emb = np.ascontiguousarray(inputs["pos_emb"].astype(f32))
    type_emb = np.ascontiguousarray(inputs["type_emb"].astype(f32))
    clf_w = np.ascontiguousarray(inputs["clf_W"].astype(f32))
    clf_b = inputs["clf_b"].astype(f32).reshape(1, NL)

    ids = inputs["input_word_ids"].astype(np.int64)
    tids = inputs["input_type_ids"].astype(np.int64)
    mask = inputs["input_mask"].astype(f32)
    valid = inputs["valid_mask"].astype(np.int64)

    def wrap16(v):
        """dma_gather index layout: idx j at [j % 16, j // 16], replicated
        across the 8 gpsimd cores' 16-partition groups."""
        blk = v.astype(np.int16).reshape(16, 16).T
        return np.ascontiguousarray(np.tile(blk, (8, 1)))

    in_maps = []
    for b in range(B):
        pm = params.copy()
        bias_k = (1.0 - mask[b]) * -10000.0
        pm[:, PC_BIASK:PC_BIASK + TT] = bias_k.reshape(TT, 128).T

        pos = np.arange(S, dtype=np.int64)
        sort_key = (1 - valid[b]) * S + pos
        order = np.argsort(sort_key, kind="stable")
        count = int(valid[b].sum())
        pT = np.zeros((S, S), f32)
        for i in range(count):
            pT[order[i], i] = 1.0
        psel = np.zeros((1, S), f32)
        psel[0, count:] = 1.0

        in_maps.append(dict(
            wq=Wq, wk=Wk, wv=Wv, wo=Wo, wi=Wi, wo2=Wo2,
            word_emb=word_emb, pos_emb=pos_emb, type_emb=type_emb,
            clf_w=clf_w, clf_b=clf_b,
            params=pm, paramsT3=paramsT3, paramsW=paramsW, selsum=selsum,
            selpair=selpair,
            rows_bf=rows,
            idw=wrap16(ids[b]), idt=wrap16(tids[b]),
            permT=pT, padsel=psel,
            ident=ident, sumsel=sumsel, ones_col=ones_col,
            ones1_bf=ones1b, onesr_bf=onesr, ones1_f=ones1f,
        ))
    return in_maps


LAST_EXEC_NS = None
LAST_RESULTS = None


def kernel(**inputs):
    global LAST_EXEC_NS
    inputs = {k: np.asarray(v) for k, v in inputs.items()}
    if TRACE:
        _ensure_ntff_hook()
    nc = _get_nc()
    in_maps = _pack_host(inputs)
    res = run_bass_kernel_spmd(nc, in_maps, core_ids=list(range(B)), trace=TRACE)
    LAST_EXEC_NS = res.exec_time_ns
    global LAST_RESULTS
    LAST_RESULTS = res.results
    out = np.stack([res.results[b]["out"] for b in range(B)], axis=0)
    return out.astype(np.float32)



# revision 46
# speedup vs baseline: 1.0456x; 1.0456x over previous
"""BERT-NER forward (12-layer BERT-base + ragged compaction + 9-class head)
as a Bass/Tile kernel on 8 Trainium2 NeuronCores.

Sharding: data-parallel over batch. Core b processes sequence b (256 tokens).
Weights are replicated (pre-cast to bf16 on host); all GEMMs run in bf16 with
f32 PSUM accumulation.

Activation layout: feature-major X^T [768 (6 tiles x 128 partitions), 256
tokens]. Attention scores are computed transposed (S^T[k, q]) so the exp'd
score matrix feeds the AV matmul directly without transposing the attention
matrix. Softmax normalization, biases, LayerNorm statistics (partition-
direction reductions) and gamma/beta application are all folded into small
auxiliary matmuls. The valid_mask compaction is a host-built permutation
matrix applied as a final (exact) fp32 matmul over the per-token probability
rows.
"""
import os
import numpy as np
import ml_dtypes
from contextlib import ExitStack

import concourse.bass as bass
import concourse.bacc as bacc
import concourse.tile as tile
from concourse import mybir
from concourse.bass_utils import run_bass_kernel_spmd

F32 = mybir.dt.float32
BF16 = mybir.dt.bfloat16
I16 = mybir.dt.int16
AF = mybir.ActivationFunctionType
ALU = mybir.AluOpType

B, S, H, L, NH, FF, V, NL = 8, 256, 768, 12, 12, 3072, 30522, 9
D = H // NH           # 64
KT = H // 128         # 6 feature tiles
TT = S // 128         # 2 token tiles
FT = FF // 128        # 24 ff tiles
SCALE = 1.0 / np.sqrt(D)
EPS_U = (H * H) * 1e-12   # eps for u = 768^2 * var

N_LAYERS = int(os.environ.get("BERT_NL", str(L)))

# ---- params packing column layout (f32, feature-major [128, PC]) ----
PC_EMB_G = 0            # 6 cols
PC_EMB_B = 6            # 6 cols
PC_BIASK = 12           # 2 cols: attention additive mask bias per k position
PC_LAYER = 14           # per layer: bq(6), bk(6), bi(24), bo(6), bo2(6) = 48
PC_STRIDE = 48
PC_TOTAL = PC_LAYER + PC_STRIDE * L
NP = NH // 2            # 6 head pairs

TRACE = False           # set by test harness for profiling runs


def _ensure_ntff_hook():
    """The agent image's antenv lacks axon_hooks; shim it so trace=True can
    drive NTFF profiling through libaxon_pjrt.so (same ABI trn_boot uses)."""
    import sys, types
    if "antenv.axon_hooks" in sys.modules:
        return
    try:
        import antenv
        mod = types.ModuleType("antenv.axon_hooks")
        state = {"hook": None}
        mod.set_axon_ntff_profile_hook = lambda h: state.__setitem__("hook", h)
        mod.get_axon_ntff_profile_hook = lambda: state["hook"]
        sys.modules["antenv.axon_hooks"] = mod
        antenv.axon_hooks = mod
        from trn_agent_boot.trn_boot import _ntff_profile_via_ctypes
        mod.set_axon_ntff_profile_hook(
            _ntff_profile_via_ctypes("/opt/axon/libaxon_pjrt.so"))
    except Exception as e:  # profiling is best-effort
        print(f"ntff hook shim failed: {e}")


def _build_nc():
    nc = bacc.Bacc("TRN2", target_bir_lowering=False)

    # ---------------- DRAM tensors ----------------
    wq = nc.dram_tensor("wq", [L * H, H], BF16, kind="ExternalInput")
    wk = nc.dram_tensor("wk", [L * H, H], BF16, kind="ExternalInput")
    wv = nc.dram_tensor("wv", [L * H, H], BF16, kind="ExternalInput")
    wo = nc.dram_tensor("wo", [L * H, H], BF16, kind="ExternalInput")
    wi = nc.dram_tensor("wi", [L * H, FF], BF16, kind="ExternalInput")
    wo2 = nc.dram_tensor("wo2", [L * FF, H], BF16, kind="ExternalInput")
    word_emb = nc.dram_tensor("word_emb", [V, H], F32, kind="ExternalInput")
    pos_emb = nc.dram_tensor("pos_emb", [S, H], F32, kind="ExternalInput")
    type_emb = nc.dram_tensor("type_emb", [2, H], F32, kind="ExternalInput")
    clf_w = nc.dram_tensor("clf_w", [H, NL], F32, kind="ExternalInput")
    clf_b = nc.dram_tensor("clf_b", [1, NL], F32, kind="ExternalInput")
    params = nc.dram_tensor("params", [128, PC_TOTAL], F32, kind="ExternalInput")
    paramsT3 = nc.dram_tensor("paramsT3", [3, 2 * L * H], BF16, kind="ExternalInput")
    paramsW = nc.dram_tensor("paramsW", [65, 2 * L * H], BF16, kind="ExternalInput")
    selsum = nc.dram_tensor("selsum", [128, NH * NH], BF16, kind="ExternalInput")
    selpair = nc.dram_tensor("selpair", [NH, NP * 128], BF16, kind="ExternalInput")
    rows_bf = nc.dram_tensor("rows_bf", [1, 3 * L * H], BF16, kind="ExternalInput")
    idw = nc.dram_tensor("idw", [128, 16], I16, kind="ExternalInput")
    idt = nc.dram_tensor("idt", [128, 16], I16, kind="ExternalInput")
    permT = nc.dram_tensor("permT", [S, S], F32, kind="ExternalInput")
    padsel = nc.dram_tensor("padsel", [1, S], F32, kind="ExternalInput")
    ident = nc.dram_tensor("ident", [128, 128], F32, kind="ExternalInput")
    sumsel = nc.dram_tensor("sumsel", [128, 4], BF16, kind="ExternalInput")
    ones_col = nc.dram_tensor("ones_col", [128, 1], BF16, kind="ExternalInput")
    ones1_bf = nc.dram_tensor("ones1_bf", [1, 128], BF16, kind="ExternalInput")
    onesr_bf = nc.dram_tensor("onesr_bf", [1, S], BF16, kind="ExternalInput")
    ones1_f = nc.dram_tensor("ones1_f", [1, 128], F32, kind="ExternalInput")

    out = nc.dram_tensor("out", [S, NL], F32, kind="ExternalOutput")
    DEBUG = os.environ.get("BERT_DEBUG", "0") == "1"
    if DEBUG:
        d_emb = nc.dram_tensor("d_emb", [H, S], F32, kind="ExternalOutput")
        d_h1 = nc.dram_tensor("d_h1", [H, S], F32, kind="ExternalOutput")
        d_ff = nc.dram_tensor("d_ff", [128, S], BF16, kind="ExternalOutput")
        d_h2 = nc.dram_tensor("d_h2", [H, S], F32, kind="ExternalOutput")
        d_pr = nc.dram_tensor("d_pr", [S, NL], F32, kind="ExternalOutput")

    with tile.TileContext(nc) as tc, ExitStack() as ctx:
        ctx.enter_context(nc.allow_low_precision(
            reason="bf16 softmax/LN normalization factors; error budget analyzed"))
        const = ctx.enter_context(tc.tile_pool(name="const", bufs=1))
        act = ctx.enter_context(tc.tile_pool(name="act", bufs=1))
        small = ctx.enter_context(tc.tile_pool(name="small", bufs=1))
        ps = ctx.enter_context(tc.tile_pool(name="ps", bufs=1, space="PSUM"))

        # ---------------- constants / params ----------------
        # gather indices first so the embedding gathers aren't queued behind
        # the bulk const/weight DMA traffic
        idw_sb = const.tile([128, 16], I16, tag="idw", name="idw_sb")
        nc.sync.dma_start(idw_sb[:], idw[:, :])
        idt_sb = const.tile([128, 16], I16, tag="idt", name="idt_sb")
        nc.sync.dma_start(idt_sb[:], idt[:, :])
        ident_sb = const.tile([128, 128], F32, tag="ident", name="ident_sb")
        nc.sync.dma_start(ident_sb[:], ident[:, :])
        sumsel_sb = const.tile([128, 4], BF16, tag="sumsel", name="sumsel_sb")
        nc.sync.dma_start(sumsel_sb[:], sumsel[:, :])
        onescol_sb = const.tile([128, 1], BF16, tag="onescol", name="onescol_sb")
        nc.sync.dma_start(onescol_sb[:], ones_col[:, :])
        ones1b_sb = const.tile([1, 128], BF16, tag="ones1b", name="ones1b_sb")
        nc.sync.dma_start(ones1b_sb[:], ones1_bf[:, :])
        onesr_sb = const.tile([1, S], BF16, tag="onesr", name="onesr_sb")
        nc.sync.dma_start(onesr_sb[:], onesr_bf[:, :])
        ones1f_sb = const.tile([1, 128], F32, tag="ones1f", name="ones1f_sb")
        nc.sync.dma_start(ones1f_sb[:], ones1_f[:, :])
        eps_sb = const.tile([128, 1], F32, tag="eps", name="eps_sb")
        nc.vector.memset(eps_sb[:], float(EPS_U))
        # rhs3 for the fused LN-apply matmul: row 0 = [r | 0], row 32 =
        # [0 | s1*r], row 64 = [0 | -1]; zero/-1 regions are written once here
        rhs3 = const.tile([65, 2 * S], BF16, tag="rhs3", name="rhs3_sb")
        nc.vector.memset(rhs3[:], 0.0)
        nc.vector.memset(rhs3[64:65, S:2 * S], -1.0)
        params_sb = const.tile([128, PC_TOTAL], F32, tag="params", name="params_sb")
        nc.sync.dma_start(params_sb[:], params[:, :])
        selsum_sb = const.tile([128, NH * NH], BF16, tag="selsum", name="selsum_sb")
        nc.sync.dma_start(selsum_sb[:], selsum[:, :])
        selpair_sb = const.tile([NH, NP * 128], BF16, tag="selpair", name="selpair_sb")
        nc.sync.dma_start(selpair_sb[:], selpair[:, :])
        clfw_sb = const.tile([128, KT * NL], F32, tag="clfw", name="clfw_sb")
        for k in range(KT):
            nc.sync.dma_start(clfw_sb[:, k * NL:(k + 1) * NL],
                              clf_w[k * 128:(k + 1) * 128, :])
        clfb_sb = const.tile([1, NL], F32, tag="clfb", name="clfb_sb")
        nc.sync.dma_start(clfb_sb[:], clf_b[:, :])

        def pcol(c):
            return params_sb[:, c:c + 1]

        # ---------------- embeddings ----------------
        h = []
        for k in range(KT):
            t = act.tile([128, S], F32, tag=f"h{k}", name=f"h_{k}", bufs=2)
            h.append(t)
        with tc.tile_pool(name="emb", bufs=1) as emb:

            xt_sb = emb.tile([128, TT * H], F32, tag="xt", name="xt_sb")
            nc.gpsimd.dma_gather(
                out_ap=xt_sb.rearrange("p (c f) -> p c f", f=H),
                in_ap=word_emb[:, :],
                idxs_ap=idw_sb[:, :],
                num_idxs=S, num_idxs_reg=S, elem_size=H,
            )
            te_sb = emb.tile([128, TT * H], F32, tag="te", name="te_sb")
            nc.gpsimd.dma_gather(
                out_ap=te_sb.rearrange("p (c f) -> p c f", f=H),
                in_ap=type_emb[:, :],
                idxs_ap=idt_sb[:, :],
                num_idxs=S, num_idxs_reg=S, elem_size=H,
            )
            pe_sb = emb.tile([128, TT * H], F32, tag="pe", name="pe_sb")
            nc.sync.dma_start(
                pe_sb.rearrange("p (c f) -> p c f", f=H),
                pos_emb[:, :].rearrange("(c p) f -> p c f", p=128),
            )
            nc.vector.tensor_add(xt_sb[:], xt_sb[:], te_sb[:])
            nc.vector.tensor_add(xt_sb[:], xt_sb[:], pe_sb[:])

            # token-major LN (stats along free dim), then transpose to feat-major
            for c in range(TT):
                xc = xt_sb[:, c * H:(c + 1) * H]
                s1 = emb.tile([128, 1], F32, tag="s1", name=f"es1_{c}", bufs=2)
                nc.vector.reduce_sum(s1[:], xc, axis=mybir.AxisListType.X)
                sq = emb.tile([128, H], F32, tag="sq", name=f"esq_{c}", bufs=2)
                s2 = emb.tile([128, 1], F32, tag="s2", name=f"es2_{c}", bufs=2)
                nc.scalar.activation(sq[:], xc, AF.Square, accum_out=s2[:])
                t1 = emb.tile([128, 1], F32, tag="t1", name=f"et1_{c}", bufs=2)
                nc.scalar.activation(t1[:], s1[:], AF.Square)
                u = emb.tile([128, 1], F32, tag="u", name=f"eu_{c}", bufs=2)
                nc.vector.scalar_tensor_tensor(u[:], s2[:], float(H), t1[:],
                                               op0=ALU.mult, op1=ALU.subtract)
                qq = emb.tile([128, 1], F32, tag="qq", name=f"eqq_{c}", bufs=2)
                nc.scalar.activation(qq[:], u[:], AF.Sqrt, bias=eps_sb[:])
                rr = emb.tile([128, 1], F32, tag="rr", name=f"err_{c}", bufs=2)
                nc.vector.reciprocal(rr[:], qq[:])
                scale0 = emb.tile([128, 1], F32, tag="scale0", name=f"esc_{c}", bufs=2)
                nc.scalar.activation(scale0[:], rr[:], AF.Copy, scale=float(H))
                bias0 = emb.tile([128, 1], F32, tag="bias0", name=f"ebi_{c}", bufs=2)
                nc.vector.scalar_tensor_tensor(bias0[:], s1[:], -1.0, rr[:],
                                               op0=ALU.mult, op1=ALU.mult)
                xln = emb.tile([128, H], F32, tag="xln", name=f"exln_{c}", bufs=2)
                nc.scalar.activation(xln[:], xc, AF.Identity,
                                     bias=bias0[:], scale=scale0[:])
                # transpose 128x128 blocks to feature-major, apply emb gamma/beta
                for k in range(KT):
                    tp = ps.tile([128, 128], F32, tag="mm", name=f"etp_{c}_{k}", bufs=4)
                    nc.tensor.transpose(tp[:], xln[:, k * 128:(k + 1) * 128],
                                        ident_sb[:])
                    nc.scalar.activation(h[k][:, c * 128:(c + 1) * 128], tp[:],
                                         AF.Identity,
                                         bias=pcol(PC_EMB_B + k),
                                         scale=pcol(PC_EMB_G + k))

        hln = []
        for k in range(KT):
            t = act.tile([128, S], BF16, tag=f"hb{k}", name=f"hb_{k}", bufs=2)
            nc.scalar.copy(t[:], h[k][:])
            hln.append(t)
        if DEBUG:
            for k in range(KT):
                nc.sync.dma_start(d_emb[k * 128:(k + 1) * 128, :], h[k][:])

        wpool = ctx.enter_context(tc.tile_pool(name="wpool", bufs=1))

        # ---------------- transformer layers ----------------
        def layernorm(h_in, hcsq, which, l):
            """which: 0 -> ln1, 1 -> ln2. Returns (h_new_tiles, hln_bf_tiles).

            Stats: one [hb|sq] concat tile per k -> 6 N=512 matmuls into a
            single [1, 2S] PSUM tile (s1 | s2). Apply: one K=3 matmul per k
            produces [sg | mb] in one PSUM bank."""
            base_t = (l * 2 + which) * H
            ptW = small.tile([65, H], BF16, tag="ptW", name=f"ptW_{l}_{which}", bufs=2)
            nc.sync.dma_start(ptW[:], paramsW[:, base_t:base_t + H])
            s12 = ps.tile([1, 2 * S], F32, tag="sc", name=f"lns_{l}_{which}", bufs=2)
            for k in range(KT):
                nc.tensor.matmul(s12[:], onescol_sb[:], hcsq[k][:],
                                 start=(k == 0), stop=(k == KT - 1))
            s1sb = small.tile([1, S], F32, tag="s1sb", name=f"s1sb_{l}_{which}",
                              bufs=2)
            nc.vector.tensor_copy(s1sb[:], s12[0:1, 0:S])
            t1 = small.tile([1, S], F32, tag="t1", name=f"t1_{l}_{which}", bufs=2)
            nc.vector.tensor_mul(t1[:], s1sb[:], s1sb[:])
            u = small.tile([1, S], F32, tag="u", name=f"u_{l}_{which}", bufs=2)
            nc.vector.scalar_tensor_tensor(u[:], s12[0:1, S:2 * S], float(H), t1[:],
                                           op0=ALU.mult, op1=ALU.subtract)
            # r lands directly in rhs3 row 0; s1*r in row 32 (base-32 write
            # is legal); row 64 holds the constant -1
            nc.scalar.activation(rhs3[0:1, 0:S], u[:], AF.Abs_reciprocal_sqrt,
                                 bias=eps_sb[0:1, :])
            nc.vector.tensor_mul(rhs3[32:33, S:2 * S], s1sb[:], rhs3[0:1, 0:S])
            h_new, hb_new = [], []
            for k in range(KT):
                sgmb = ps.tile([128, 2 * S], F32, tag="sc",
                               name=f"sgmb_{l}_{which}_{k}", bufs=2)
                nc.tensor.matmul(sgmb[:], ptW[:, k * 128:(k + 1) * 128], rhs3[:])
                tmp = small.tile([128, S], F32, tag="lntmp",
                                 name=f"lnt_{l}_{which}_{k}", bufs=2)
                nc.vector.tensor_mul(tmp[:], h_in[k][:], sgmb[:, 0:S])
                hn = act.tile([128, S], F32, tag=f"h{k}", name=f"h_{l}_{which}_{k}",
                              bufs=2)
                nc.vector.tensor_sub(hn[:], tmp[:], sgmb[:, S:2 * S])
                hb = act.tile([128, S], BF16, tag=f"hb{k}",
                              name=f"hbn_{l}_{which}_{k}", bufs=2)
                nc.scalar.copy(hb[:], hn[:])
                h_new.append(hn)
                hb_new.append(hb)
            # preload the table the NEXT scalar table-function needs (gelu
            # after ln1, next layer's exp after ln2) AFTER the hb casts so the
            # load hides under the following GEMM phase
            dum2 = small.tile([1, 1], F32, tag="dummy", name=f"dum2_{l}_{which}",
                              bufs=2)
            nc.scalar.activation(dum2[:], rhs3[0:1, 0:1],
                                 AF.Gelu if which == 0 else AF.Exp)
            return h_new, hb_new

        for l in range(N_LAYERS):
            # ---- weight panels ----
            wq_p, wk_p, wv_p, wo_p = [], [], [], []
            for k in range(KT):
                t = wpool.tile([128, H], BF16, tag="wp768", name=f"wq_{l}_{k}", bufs=36)
                nc.sync.dma_start(t[:], wq[l * H + k * 128: l * H + (k + 1) * 128, :])
                wq_p.append(t)
            for k in range(KT):
                t = wpool.tile([128, H], BF16, tag="wp768", name=f"wk_{l}_{k}", bufs=36)
                nc.sync.dma_start(t[:], wk[l * H + k * 128: l * H + (k + 1) * 128, :])
                wk_p.append(t)
            for k in range(KT):
                t = wpool.tile([128, H], BF16, tag="wp768", name=f"wv_{l}_{k}", bufs=36)
                nc.sync.dma_start(t[:], wv[l * H + k * 128: l * H + (k + 1) * 128, :])
                wv_p.append(t)

            pbase = PC_LAYER + PC_STRIDE * l

            # ---- Q^T, K^T per feature-pair, scores + exp interleaved so the
            # scalar-engine exps hide under the Q/K GEMMs ----
            q_bf, k_bf = [], []
            e_bf = [[None] * TT for _ in range(NP)]
            for m in range(KT):
                accq = ps.tile([128, S], F32, tag="mm", name=f"q_{l}_{m}", bufs=4)
                for k in range(KT):
                    nc.tensor.matmul(accq[:], wq_p[k][:, m * 128:(m + 1) * 128],
                                     hln[k][:], start=(k == 0), stop=(k == KT - 1))
                qo = act.tile([128, S], BF16, tag=f"qk0_{m}",
                              name=f"qko_{l}_{m}", bufs=1)
                nc.vector.tensor_scalar_add(qo[:], accq[:], pcol(pbase + m))
                q_bf.append(qo)
                acck = ps.tile([128, S], F32, tag="mm", name=f"k_{l}_{m}", bufs=4)
                for k in range(KT):
                    nc.tensor.matmul(acck[:], wk_p[k][:, m * 128:(m + 1) * 128],
                                     hln[k][:], start=(k == 0), stop=(k == KT - 1))
                ko = act.tile([128, S], BF16, tag=f"qk1_{m}",
                              name=f"qko1_{l}_{m}", bufs=1)
                nc.vector.tensor_scalar_add(ko[:], acck[:], pcol(pbase + 6 + m))
                k_bf.append(ko)
                # scores for head pair m: two heads concurrent via row groups.
                # One PSUM bank per accumulation group (start zeroes the whole
                # bank), exp'd halves land in one wide SBUF tile.
                for kt in range(TT):
                    e = act.tile([128, 2 * S], BF16, tag=f"e{m}_{kt}",
                                 name=f"e_{l}_{m}_{kt}", bufs=1)
                    for sub in range(2):
                        sc = ps.tile([128, S], F32, tag="mm",
                                     name=f"sc_{l}_{m}_{kt}_{sub}", bufs=4)
                        nc.tensor.matmul(
                            sc[:],
                            ko[sub * D:(sub + 1) * D, kt * 128:(kt + 1) * 128],
                            qo[sub * D:(sub + 1) * D, :])
                        nc.scalar.activation(e[:, sub * S:(sub + 1) * S], sc[:],
                                             AF.Exp, bias=pcol(PC_BIASK + kt),
                                             scale=float(SCALE))
                    e_bf[m][kt] = e

            # ---- V (X^T stationary -> token-major V); bv folded into bo ----
            v_bf = []
            for mt in range(TT):
                accA = ps.tile([128, 512], F32, tag="sc", name=f"vA_{l}_{mt}", bufs=2)
                accB = ps.tile([128, S], F32, tag="mm", name=f"vB_{l}_{mt}", bufs=4)
                for k in range(KT):
                    nc.tensor.matmul(accA[:],
                                     hln[k][:, mt * 128:(mt + 1) * 128],
                                     wv_p[k][:, 0:512],
                                     start=(k == 0), stop=(k == KT - 1))
                    nc.tensor.matmul(accB[:],
                                     hln[k][:, mt * 128:(mt + 1) * 128],
                                     wv_p[k][:, 512:H],
                                     start=(k == 0), stop=(k == KT - 1))
                vb = act.tile([128, H], BF16, tag=f"v{mt}", name=f"vb_{l}_{mt}", bufs=1)
                nc.scalar.copy(vb[:, 0:512], accA[:])
                nc.scalar.copy(vb[:, 512:H], accB[:])
                v_bf.append(vb)

            # ---- softmax sums first (selector-accumulated into one [NH, S]
            # tile) so the reciprocal runs on DVE while PE does the AV matmuls ----
            sums_ps = ps.tile([NH, S], F32, tag="sc", name=f"sums_{l}", bufs=2)
            for pair in range(NP):
                for kt in range(TT):
                    for sub in range(2):
                        hh = 2 * pair + sub
                        nc.tensor.matmul(
                            sums_ps[:],
                            selsum_sb[:, hh * NH:(hh + 1) * NH],
                            e_bf[pair][kt][:, sub * S:(sub + 1) * S],
                            start=(pair == 0 and kt == 0 and sub == 0),
                            stop=(pair == NP - 1 and kt == TT - 1 and sub == 1))
            rc_bf = small.tile([NH, S], BF16, tag="rcb", name=f"rcb_{l}", bufs=2)
            nc.vector.reciprocal(rc_bf[:], sums_ps[:])

            # unnormalized AV (PSUM evac on scalar; DVE is busy with the
            # reciprocal)
            cb0 = []
            for pair in range(NP):
                c0 = act.tile([128, S], BF16, tag=f"cb0_{pair}",
                              name=f"cb0_{l}_{pair}", bufs=1)
                for sub in range(2):
                    hh = 2 * pair + sub
                    cps = ps.tile([64, S], F32, tag="cps",
                                  name=f"cps_{l}_{pair}_{sub}", bufs=2)
                    for kt in range(TT):
                        nc.tensor.matmul(
                            cps[:],
                            v_bf[kt][:, hh * D:(hh + 1) * D],
                            e_bf[pair][kt][:, sub * S:(sub + 1) * S],
                            start=(kt == 0), stop=(kt == TT - 1))
                    nc.scalar.copy(c0[sub * D:(sub + 1) * D, :], cps[:])
                cb0.append(c0)

            ctx_bf = []
            for pair in range(NP):
                rb = ps.tile([128, S], F32, tag="cps", name=f"rb_{l}_{pair}", bufs=2)
                nc.tensor.matmul(rb[:], selpair_sb[:, pair * 128:(pair + 1) * 128],
                                 rc_bf[:])
                rbs = small.tile([128, S], BF16, tag="rbs",
                                 name=f"rbs_{l}_{pair}", bufs=3)
                nc.vector.tensor_copy(rbs[:], rb[:])
                cb = act.tile([128, S], BF16, tag=f"ctx{pair}",
                              name=f"ctx_{l}_{pair}", bufs=1)
                nc.vector.tensor_mul(cb[:], cb0[pair][:], rbs[:])
                ctx_bf.append(cb)

            # ---- O projection + residual ----
            for k in range(KT):
                t = wpool.tile([128, H], BF16, tag="wp768", name=f"wo_{l}_{k}", bufs=36)
                nc.sync.dma_start(t[:], wo[l * H + k * 128: l * H + (k + 1) * 128, :])
                wo_p.append(t)
            h_res, hcsq1 = [], []
            for m in range(KT):
                acc = ps.tile([128, S], F32, tag="mm", name=f"o_{l}_{m}", bufs=4)
                for k in range(KT):
                    nc.tensor.matmul(acc[:], wo_p[k][:, m * 128:(m + 1) * 128],
                                     ctx_bf[k][:], start=(k == 0),
                                     stop=(k == KT - 1))
                hr = small.tile([128, S], F32, tag=f"hr{m}", name=f"hr_{l}_{m}", bufs=1)
                nc.vector.scalar_tensor_tensor(hr[:], acc[:], pcol(pbase + 36 + m),
                                               h[m][:], op0=ALU.add, op1=ALU.add)
                h_res.append(hr)
                hc = small.tile([128, 2 * S], BF16, tag="hcsq",
                                name=f"hcsq_{l}_0_{m}", bufs=7)
                nc.vector.tensor_copy(hc[:, 0:S], hr[:])
                nc.vector.tensor_mul(hc[:, S:2 * S], hr[:], hr[:])
                hcsq1.append(hc)
                if m == 0:
                    # anchored dummy: preloads the Abs_reciprocal_sqrt table
                    # during the remaining O-proj matmuls (the data dependency
                    # pins it here in the schedule)
                    dm = small.tile([1, 1], F32, tag="dummy",
                                    name=f"dma_{l}_0", bufs=2)
                    nc.scalar.activation(dm[:], hc[0:1, 0:1],
                                         AF.Abs_reciprocal_sqrt)

            h, hln = layernorm(h_res, hcsq1, 0, l)
            if DEBUG and l == 0:
                for k in range(KT):
                    nc.sync.dma_start(d_h1[k * 128:(k + 1) * 128, :], h[k][:])

            # ---- FF1 (Wi stationary) + gelu ----
            wi_p = []
            for k in range(KT):
                t = wpool.tile([128, FF], BF16, tag="wp3072", name=f"wi_{l}_{k}",
                               bufs=7)
                nc.sync.dma_start(t[:], wi[l * H + k * 128: l * H + (k + 1) * 128, :])
                wi_p.append(t)
            ff_bf = []
            for m in range(FT):
                acc = ps.tile([128, S], F32, tag="mm", name=f"ff1_{l}_{m}", bufs=4)
                for k in range(KT):
                    nc.tensor.matmul(acc[:], wi_p[k][:, m * 128:(m + 1) * 128],
                                     hln[k][:], start=(k == 0), stop=(k == KT - 1))
                fb = act.tile([128, S], BF16, tag=f"ff{m}", name=f"ff_{l}_{m}", bufs=1)
                nc.scalar.activation(fb[:], acc[:], AF.Gelu,
                                     bias=pcol(pbase + 12 + m))
                ff_bf.append(fb)

            if DEBUG and l == 0:
                nc.sync.dma_start(d_ff[:, :], ff_bf[0][:])

            # ---- FF2 (m-outer, K-contiguous per output tile) + residual ----
            wo2_p = []
            for k in range(FT):
                t = wpool.tile([128, H], BF16, tag="wp768", name=f"wo2_{l}_{k}",
                               bufs=36)
                nc.sync.dma_start(t[:], wo2[l * FF + k * 128: l * FF + (k + 1) * 128, :])
                wo2_p.append(t)
            h_res2, hcsq2 = [], []
            for m in range(KT):
                acc = ps.tile([128, S], F32, tag="mm", name=f"ff2_{l}_{m}", bufs=4)
                for k in range(FT):
                    nc.tensor.matmul(acc[:], wo2_p[k][:, m * 128:(m + 1) * 128],
                                     ff_bf[k][:], start=(k == 0),
                                     stop=(k == FT - 1))
                hr = small.tile([128, S], F32, tag=f"hr{m}", name=f"hr2_{l}_{m}",
                                bufs=1)
                nc.vector.scalar_tensor_tensor(hr[:], acc[:], pcol(pbase + 42 + m),
                                               h[m][:], op0=ALU.add, op1=ALU.add)
                h_res2.append(hr)
                hc = small.tile([128, 2 * S], BF16, tag="hcsq",
                                name=f"hcsq_{l}_1_{m}", bufs=7)
                nc.vector.tensor_copy(hc[:, 0:S], hr[:])
                nc.vector.tensor_mul(hc[:, S:2 * S], hr[:], hr[:])
                hcsq2.append(hc)
                if m == 0:
                    dm = small.tile([1, 1], F32, tag="dummy",
                                    name=f"dma_{l}_1", bufs=2)
                    nc.scalar.activation(dm[:], hc[0:1, 0:1],
                                         AF.Abs_reciprocal_sqrt)

            h, hln = layernorm(h_res2, hcsq2, 1, l)
            if DEBUG and l == 0:
                for k in range(KT):
                    nc.sync.dma_start(d_h2[k * 128:(k + 1) * 128, :], h[k][:])

        # ---------------- classifier + softmax + compaction ----------------
        permT_sb = []
        for kt in range(TT):
            for mt in range(TT):
                t = small.tile([128, 128], F32, tag=f"permT{kt}_{mt}",
                               name=f"permT_{kt}_{mt}", bufs=1)
                nc.sync.dma_start(t[:], permT[kt * 128:(kt + 1) * 128,
                                              mt * 128:(mt + 1) * 128])
                permT_sb.append(t)
        padsel_sb = small.tile([1, S], F32, tag="padsel", name="padsel_sb", bufs=1)
        nc.sync.dma_start(padsel_sb[:], padsel[:, :])

        # pad row = softmax(clf_b)
        pmx = small.tile([1, 1], F32, tag="pmx", name="pmx", bufs=1)
        nc.vector.reduce_max(pmx[:], clfb_sb[:], axis=mybir.AxisListType.X,
                             negate=True)
        pex = small.tile([1, NL], F32, tag="pex", name="pex", bufs=1)
        psm = small.tile([1, 1], F32, tag="psm", name="psm", bufs=1)
        nc.scalar.activation(pex[:], clfb_sb[:], AF.Exp, bias=pmx[:],
                             accum_out=psm[:])
        prs = small.tile([1, 1], F32, tag="prs", name="prs", bufs=1)
        nc.vector.reciprocal(prs[:], psm[:])
        ppr = small.tile([1, NL], F32, tag="ppr", name="ppr", bufs=1)
        nc.vector.tensor_scalar_mul(ppr[:], pex[:], prs[:])

        probs = []
        for mt in range(TT):
            acc = ps.tile([128, NL], F32, tag="mm", name=f"clf_{mt}", bufs=4)
            for k in range(KT):
                nc.tensor.matmul(acc[:], h[k][:, mt * 128:(mt + 1) * 128],
                                 clfw_sb[:, k * NL:(k + 1) * NL],
                                 start=(k == 0), stop=False)
            nc.tensor.matmul(acc[:], ones1f_sb[:], clfb_sb[:],
                             start=False, stop=True)
            mx = small.tile([128, 1], F32, tag="mx", name=f"mx_{mt}", bufs=2)
            nc.vector.reduce_max(mx[:], acc[:], axis=mybir.AxisListType.X,
                                 negate=True)
            ex = small.tile([128, NL], F32, tag="ex", name=f"ex_{mt}", bufs=2)
            sm = small.tile([128, 1], F32, tag="sm", name=f"sm_{mt}", bufs=2)
            nc.scalar.activation(ex[:], acc[:], AF.Exp, bias=mx[:],
                                 accum_out=sm[:])
            rs = small.tile([128, 1], F32, tag="rs", name=f"rs_{mt}", bufs=2)
            nc.vector.reciprocal(rs[:], sm[:])
            pr = small.tile([128, NL], F32, tag=f"pr{mt}", name=f"pr_{mt}", bufs=1)
            nc.vector.tensor_scalar_mul(pr[:], ex[:], rs[:])
            probs.append(pr)

        if DEBUG:
            for mt in range(TT):
                nc.sync.dma_start(d_pr[mt * 128:(mt + 1) * 128, :], probs[mt][:])

        # compacted output rows: out[i] = probs[order[i]] (i < count) else pad
        for mt in range(TT):
            acc = ps.tile([128, NL], F32, tag="mm", name=f"cmp_{mt}", bufs=4)
            for kt in range(TT):
                nc.tensor.matmul(acc[:], permT_sb[kt * TT + mt][:], probs[kt][:],
                                 start=(kt == 0), stop=False)
            nc.tensor.matmul(acc[:], padsel_sb[0:1, mt * 128:(mt + 1) * 128],
                             ppr[:], start=False, stop=True)
            osb = small.tile([128, NL], F32, tag=f"osb{mt}", name=f"osb_{mt}", bufs=1)
            nc.scalar.copy(osb[:], acc[:])
            nc.sync.dma_start(out[mt * 128:(mt + 1) * 128, :], osb[:])

    nc.finalize()
    return nc


_NC_CACHE = {}


def _get_nc():
    key = N_LAYERS
    if key not in _NC_CACHE:
        _NC_CACHE[key] = _build_nc()
    return _NC_CACHE[key]


def _pack_host(inputs):
    """Builds per-core in_maps (host-side sharding + descriptor prep)."""
    f32 = np.float32
    bf16 = ml_dtypes.bfloat16

    Wq = np.ascontiguousarray(inputs["Wq"].astype(bf16).reshape(L * H, H))
    Wk = np.ascontiguousarray(inputs["Wk"].astype(bf16).reshape(L * H, H))
    Wv = np.ascontiguousarray(inputs["Wv"].astype(bf16).reshape(L * H, H))
    Wo = np.ascontiguousarray(inputs["Wo"].astype(bf16).reshape(L * H, H))
    Wi = np.ascontiguousarray(inputs["Wi"].astype(bf16).reshape(L * H, FF))
    Wo2 = np.ascontiguousarray(inputs["Wo2"].astype(bf16).reshape(L * FF, H))

    # params (feature-major per-partition columns)
    params = np.zeros((128, PC_TOTAL), f32)
    params[:, PC_EMB_G:PC_EMB_G + 6] = inputs["emb_ln_g"].reshape(6, 128).T
    params[:, PC_EMB_B:PC_EMB_B + 6] = inputs["emb_ln_b"].reshape(6, 128).T
    for l in range(L):
        base = PC_LAYER + PC_STRIDE * l
        params[:, base:base + 6] = inputs["bq"][l].reshape(6, 128).T
        params[:, base + 6:base + 12] = inputs["bk"][l].reshape(6, 128).T
        params[:, base + 12:base + 36] = inputs["bi"][l].reshape(24, 128).T
        bo_folded = (inputs["bo"][l] + inputs["bv"][l].astype(np.float64) @
                     inputs["Wo"][l].astype(np.float64)).astype(f32)
        params[:, base + 36:base + 42] = bo_folded.reshape(6, 128).T
        params[:, base + 42:base + 48] = inputs["bo2"][l].reshape(6, 128).T

    # paramsT3 rows: [768*gamma, gamma, beta]
    paramsT3 = np.zeros((3, 2 * L * H), f32)
    for l in range(L):
        for which, (g, b) in enumerate(
                [(inputs["ln1_g"][l], inputs["ln1_b"][l]),
                 (inputs["ln2_g"][l], inputs["ln2_b"][l])]):
            c0 = (l * 2 + which) * H
            paramsT3[0, c0:c0 + H] = g * float(H)
            paramsT3[1, c0:c0 + H] = g
            paramsT3[2, c0:c0 + H] = b
    paramsT3 = paramsT3.astype(bf16)
    # K=65 padded LN-apply weights: row 0 = H*gamma, row 32 = gamma, row 64 =
    # beta (other rows zero, matching rhs3's sparse row layout)
    paramsW = np.zeros((65, 2 * L * H), f32)
    paramsW[0] = np.asarray(paramsT3[0], dtype=f32)
    paramsW[32] = np.asarray(paramsT3[1], dtype=f32)
    paramsW[64] = np.asarray(paramsT3[2], dtype=f32)
    paramsW = paramsW.astype(bf16)

    # softmax-sum selector: block hh is [128, NH] with column hh all-ones
    selsum = np.zeros((128, NH * NH), f32)
    for hh in range(NH):
        selsum[:, hh * NH + hh] = 1.0
    selsum = selsum.astype(bf16)
    # reciprocal broadcast selector: block `pair` maps rc rows (2p, 2p+1)
    # to output partitions [0:64), [64:128)
    selpair = np.zeros((NH, (NH // 2) * 128), f32)
    for pair in range(NH // 2):
        selpair[2 * pair, pair * 128:pair * 128 + 64] = 1.0
        selpair[2 * pair + 1, pair * 128 + 64:pair * 128 + 128] = 1.0
    selpair = selpair.astype(bf16)

    # rows_bf: [unused | bo' | bo2] blocks per layer, single partition row.
    # bv is folded into bo: attention ctx rows are normalized (sum to 1), so
    # ctx_with_bias = ctx_norm + 1*bv and (ctx+1*bv) @ Wo = ctx @ Wo + bv @ Wo.
    rows = np.zeros((1, 3 * L * H), f32)
    for l in range(L):
        bo_folded = inputs["bo"][l] + inputs["bv"][l].astype(np.float64) @ \
            inputs["Wo"][l].astype(np.float64)
        rows[0, 3 * l * H + 1 * H:3 * l * H + 2 * H] = bo_folded.astype(f32)
        rows[0, 3 * l * H + 2 * H:3 * l * H + 3 * H] = inputs["bo2"][l]
    rows = rows.astype(bf16)

    ident = np.eye(128, dtype=f32)
    sumsel = np.zeros((128, 4), f32)
    sumsel[:, 0] = 1.0   # S1 -> row 0
    sumsel[:, 3] = 1.0   # S2 -> row 1
    sumsel = sumsel.astype(bf16)
    ones_col = np.ones((128, 1), bf16)
    ones1b = np.ones((1, 128), bf16)
    onesr = np.ones((1, S), bf16)
    ones1f = np.ones((1, 128), f32)

    word_emb = np.ascontiguousarray(inputs["word_emb"].astype(f32))
    pos_emb = np.ascontiguousarray(inputs["pos_emb"].astype(f32))
    type_emb = np.ascontiguousarray(inputs["type_emb"].astype(f32))
    clf_w = np.ascontiguousarray(inputs["clf_W"].astype(f32))
    clf_b = inputs["clf_b"].astype(f32).reshape(1, NL)

    ids = inputs["input_word_ids"].astype(np.int64)
    tids = inputs["input_type_ids"].astype(np.int64)
    mask = inputs["input_mask"].astype(f32)
    valid = inputs["valid_mask"].astype(np.int64)

    def wrap16(v):
        """dma_gather index layout: idx j at [j % 16, j // 16], replicated
        across the 8 gpsimd cores' 16-partition groups."""
        blk = v.astype(np.int16).reshape(16, 16).T
        return np.ascontiguousarray(np.tile(blk, (8, 1)))

    in_maps = []
    for b in range(B):
        pm = params.copy()
        bias_k = (1.0 - mask[b]) * -10000.0
        pm[:, PC_BIASK:PC_BIASK + TT] = bias_k.reshape(TT, 128).T

        pos = np.arange(S, dtype=np.int64)
        sort_key = (1 - valid[b]) * S + pos
        order = np.argsort(sort_key, kind="stable")
        count = int(valid[b].sum())
        pT = np.zeros((S, S), f32)
        for i in range(count):
            pT[order[i], i] = 1.0
        psel = np.zeros((1, S), f32)
        psel[0, count:] = 1.0

        in_maps.append(dict(
            wq=Wq, wk=Wk, wv=Wv, wo=Wo, wi=Wi, wo2=Wo2,
            word_emb=word_emb, pos_emb=pos_emb, type_emb=type_emb,
            clf_w=clf_w, clf_b=clf_b,
            params=pm, paramsT3=paramsT3, paramsW=paramsW, selsum=selsum,
            selpair=selpair,
            rows_bf=rows,
            idw=wrap16(ids[b]), idt=wrap16(tids[b]),
            permT=pT, padsel=psel,
            ident=ident, sumsel=sumsel, ones_col=ones_col,
            ones1_bf=ones1b, onesr_bf=onesr, ones1_f=ones1f,
        ))
    return in_maps


LAST_EXEC_NS = None
LAST_RESULTS = None


def kernel(**inputs):
    global LAST_EXEC_NS
    inputs = {k: np.asarray(v) for k, v in inputs.items()}
    if TRACE:
        _ensure_ntff_hook()
    nc = _get_nc()
    in_maps = _pack_host(inputs)
    res = run_bass_kernel_spmd(nc, in_maps, core_ids=list(range(B)), trace=TRACE)
    LAST_EXEC_NS = res.exec_time_ns
    global LAST_RESULTS
    LAST_RESULTS = res.results
    out = np.stack([res.results[b]["out"] for b in range(B)], axis=0)
    return out.astype(np.float32)

